# revision 77
# baseline (speedup 1.0000x reference)
# DETR multi-head dot-product attention for Trainium2 (Bass/Tile), 8 NeuronCores.
#
# Problem (hardcoded): B=4, S=1024, D=1024, H=16, HD=64, f32.
#   q = (inputs_q + pos_emb_q) @ wq + bq;  q /= sqrt(HD)
#   k = (inputs_kv + pos_emb_k) @ wk + bk
#   v = (inputs_kv + pos_emb_v) @ wv + bv          (bv == 0 by problem spec)
#   attn = softmax(q k^T + key_padding_bias); out = (attn v) @ wo + bo
#
# Sharding: 8 cores = 4 batches x 2 head-groups of 8 heads. Each core computes
# its batch's projections restricted to its head-group's features (512 of 1024),
# full attention for its 8 heads, and a partial output projection. The host
# sums the two head-group partials per batch.
#
# The host pre-adds the positional embeddings (qin = inputs_q + pos_emb_q etc.)
# and ships the three activation tensors feature-major ([D, S]) in bf16, which
# more than halves HBM traffic vs a five-f32-array variant and removes all
# input adds from the DVE. All matmuls run in bf16 (1 PE cycle/row); PSUM
# accumulation stays f32.
#
# Matmul convention: out[M,N] = lhsT[K,M].T @ rhs[K,N], contraction over the
# partition dim K; cost is N cycles. Softmax runs over the partition axis of
# transposed logits L^T[S_k, S_q]. AV runs in the "natural" orientation
# out[q, hd] = sum_k P^T[k,q] V[k,hd] so all 128 output partitions are used
# (65-row moving operand: HD columns + one denominator column fed by a
# mask-valued extra column of V — masked keys contribute 0 to numerator and
# denominator, which is exactly the -1e10 key-padding bias). The per-query
# denominator then sits in the PSUM free dim: reciprocal + per-partition
# scale on DVE, transpose back to feature-major via the PE (128 cycles), and
# a DVE copy into x^T for the output projection.
#
# Schedule: the DMA queue ladder (identity, wk m0, kin halves, biases,
# wq m0, qin) lets the K projections start ~5us in and the first logits
# ~14us in. The remaining K/Q/V projection chains are injected between the
# logits chunk-pairs of the early attention slots (all of V strictly before
# the first AV chain; each kt/qt m-chunk one slot before its first reader).
# AVs trail the exp stream and drain two per slot in (even,odd) head pairs
# that share 128-wide transposes; logits pairs interleave with AV chains so
# the PE never waits on the exp stream's 2-deep PSUM ring. The sh0 output
# projection rides in slots 12-15 and only the last AV + the sh1 output
# projection trail the final exp; in that drain the last AV runs first so
# its DVE epilogue overlaps the remaining sh0 chains on the PE, the (dead)
# logits PSUM pool doubles as extra out-projection accumulators, and the
# bias epilogues alternate DVE/ACT, so the final chains run back-to-back.

import sys

for _p in ("/opt/trn_rl_repo", "/root/.axon_site/_ro/trn_rl_repo"):
    if _p not in sys.path:
        sys.path.append(_p)

import numpy as np
import ml_dtypes

import concourse.bass as bass
import concourse.mybir as mybir
import concourse.tile as tile
from concourse import bacc
from concourse.bass_utils import run_bass_kernel_spmd

B, S, D = 4, 1024, 1024
H, HD = 16, 64
F = 512          # features per head-group core (8 heads * 64)
NH = 8           # heads per core
NEG_BIG = -1e10
P = 128          # partitions
KC = D // P      # contraction chunks for the input projections (8)
SC = S // P      # sequence chunks (8)
SH = 512         # S-half (moving-operand free dim)
QC = SH // P     # query chunks per half (4)

f32 = mybir.dt.float32
bf16 = mybir.dt.bfloat16
bf16_np = ml_dtypes.bfloat16


def build_program(repeat=1):
    nc = bacc.Bacc("TRN2", target_bir_lowering=False, debug=False)

    qin_d = nc.dram_tensor("qin", [D, S], bf16, kind="ExternalInput")
    kin_d = nc.dram_tensor("kin", [D, S], bf16, kind="ExternalInput")
    vin_d = nc.dram_tensor("vin", [D, S], bf16, kind="ExternalInput")
    # wq/wk ship pre-arranged m-major ([P, m, k, 128]) so the m0 slice the
    # head projections need first is a contiguous (fast) DMA
    wq_d = nc.dram_tensor("wq", [P, 4, KC, P], bf16, kind="ExternalInput")
    wk_d = nc.dram_tensor("wk", [P, 4, KC, P], bf16, kind="ExternalInput")
    wv_d = nc.dram_tensor("wv", [D, F], bf16, kind="ExternalInput")
    wo_d = nc.dram_tensor("wo", [F, D], bf16, kind="ExternalInput")
    # packed small operands: f32 [bk | bq | bo | mk], bf16 transpose identity
    smf_d = nc.dram_tensor("smf", [P, 24], f32, kind="ExternalInput")
    smb_d = nc.dram_tensor("smb", [P, P], bf16, kind="ExternalInput")
    out_d = nc.dram_tensor("out_t", [D, S], bf16, kind="ExternalOutput")

    with tile.TileContext(nc) as tc:
        with (
            tc.tile_pool(name="acts", bufs=6) as acts_pool,
            tc.tile_pool(name="wmat", bufs=3) as w_pool,
            tc.tile_pool(name="persist", bufs=1) as persist,
            tc.tile_pool(name="pbuf", bufs=7) as p_pool,
            tc.tile_pool(name="stg", bufs=2) as stg_pool,
            tc.tile_pool(name="rcp", bufs=2) as rc_pool,
            tc.tile_pool(name="outb", bufs=4) as out_pool,
            tc.tile_pool(name="pslg", bufs=2, space=bass.MemorySpace.PSUM) as pslg,
            tc.tile_pool(name="ps", bufs=2, space=bass.MemorySpace.PSUM) as ps,
            tc.tile_pool(name="psav", bufs=2, space=bass.MemorySpace.PSUM) as psav,
        ):
            # ---- persistent tiles ----
            qt = persist.tile([P, 4, S], bf16, tag="qt")     # Q^T  [feature, s]
            kt = persist.tile([P, 4, S], bf16, tag="kt")     # K^T  [feature, s]
            xt = persist.tile([P, 4, S], bf16, tag="xt")     # attn-out^T, normalized
            # V in natural layout [s, head, hd] with a mask column per head.
            vsb = persist.tile([P, SC, NH, HD + 1], bf16, tag="vsb")
            smf = persist.tile([P, 24], f32, tag="smf")
            smb = persist.tile([P, P], bf16, tag="smb")
            bk_sb = smf[:, 0:4]
            bq_sb = smf[:, 4:8]
            bo_sb = smf[:, 8:16]
            mk_sb = smf[:, 16:24]
            ident_sb = smb[:, :]

            for _rep in range(repeat):
                wk_sb = w_pool.tile([P, 4, KC, P], bf16, tag="w")
                nc.sync.dma_start(wk_sb[:, 0], wk_d[:, 0])

                def emit_load(src, sh, split=1):
                    # split k-chunks across DMAs so dependent projection
                    # chains can start on the first half
                    t = acts_pool.tile([P, KC, SH], bf16, tag="acts")
                    step = KC // split
                    for c0 in range(0, KC, step):
                        nc.sync.dma_start(
                            t[:, c0:c0 + step, :],
                            src[c0 * P:(c0 + step) * P,
                                sh * SH:(sh + 1) * SH].rearrange(
                                    "(c p) s -> p c s", p=P))
                    return t

                kin0 = emit_load(kin_d, 0, split=4)
                kin1 = emit_load(kin_d, 1, split=4)
                nc.sync.dma_start(smf[:], smf_d[:])
                wq_sb = w_pool.tile([P, 4, KC, P], bf16, tag="w")
                nc.sync.dma_start(wq_sb[:, 0], wq_d[:, 0])
                qin0 = emit_load(qin_d, 0, split=4)
                # ident is first needed by the transposes ~55us in
                nc.sync.dma_start(smb[:], smb_d[:])
                # V's mask column via 8 tiny ACT fills (a DMA scatter into the
                # strided column would cost 8192 2-byte descriptors ~= 3.6us)
                for sc in range(SC):
                    nc.scalar.activation(
                        vsb[:, sc, :, HD], smf[:, 0:NH],
                        mybir.ActivationFunctionType.Identity,
                        bias=mk_sb[:, sc:sc + 1], scale=0.0)
                nc.sync.dma_start(wk_sb[:, 1:4], wk_d[:, 1:4])
                nc.sync.dma_start(wq_sb[:, 1:4], wq_d[:, 1:4])
                wv_sb = w_pool.tile([P, KC, F], bf16, tag="w")
                nc.sync.dma_start(
                    wv_sb[:], wv_d[:].rearrange("(k p) f -> p k f", p=P))
                vin0 = emit_load(vin_d, 0)
                vin1 = emit_load(vin_d, 1)

                def emit_kchain(sh, kin, m, act=False):
                    # K^T = (wk^T kin^T) + bk; head-phase epilogues ride the
                    # idle ACT engine so the first logits don't wait on DVE
                    acc = ps.tile([P, SH], f32, tag="ps")
                    for k in range(KC):
                        nc.tensor.matmul(
                            acc[:],
                            wk_sb[:, m, k, :],
                            kin[:, k, :],
                            start=(k == 0), stop=(k == KC - 1))
                    dst = kt[:, m, sh * SH:(sh + 1) * SH]
                    if act:
                        nc.scalar.activation(
                            dst, acc[:],
                            mybir.ActivationFunctionType.Identity,
                            bias=bk_sb[:, m:m + 1])
                    else:
                        nc.vector.tensor_scalar_add(dst, acc[:], bk_sb[:, m:m + 1])

                def emit_vchain(sh, vin, s):
                    # V in natural [s, f] layout: lhsT = vin chunk, rhs = wv;
                    # scaled by the padding mask (exact equiv of -1e10 bias)
                    sc = sh * 4 + s
                    acc = ps.tile([P, SH], f32, tag="ps")
                    for k in range(KC):
                        nc.tensor.matmul(
                            acc[:],
                            vin[:, k, s * P:(s + 1) * P],
                            wv_sb[:, k, :],
                            start=(k == 0), stop=(k == KC - 1))
                    nc.vector.tensor_scalar(
                        vsb[:, sc, :, 0:HD],
                        acc[:].rearrange("p (h d) -> p h d", d=HD),
                        mk_sb[:, sc:sc + 1], None,
                        op0=mybir.AluOpType.mult)

                def emit_qchain(sh, qin, m, act=False, head=False):
                    # the head Q chain borrows a (pre-logits, idle) lg tile so
                    # it never waits the ps ring's DVE bias-add of K(m0)
                    if head:
                        lgacc = pslg.tile([P, 2, SH], f32, tag="lg")
                        acc = lgacc[:, 0, :]
                    else:
                        psacc = ps.tile([P, SH], f32, tag="ps")
                        acc = psacc[:]
                    for k in range(KC):
                        nc.tensor.matmul(
                            acc,
                            wq_sb[:, m, k, :],
                            qin[:, k, :],
                            start=(k == 0), stop=(k == KC - 1))
                    dst = qt[:, m, sh * SH:(sh + 1) * SH]
                    if act:
                        nc.scalar.activation(
                            dst, acc,
                            mybir.ActivationFunctionType.Identity,
                            bias=bq_sb[:, m:m + 1])
                    else:
                        nc.vector.tensor_scalar_add(dst, acc, bq_sb[:, m:m + 1])

                def emit_qk_pair(sh, h, pt, cp):
                    """logits + exp for one key-chunk-pair of one head/half."""
                    po = (h % 2) * HD
                    mq = h // 2
                    lg = pslg.tile([P, 2, SH], f32, tag="lg")
                    for i in range(2):
                        c = 2 * cp + i
                        nc.tensor.matmul(
                            lg[:, i, :],
                            kt[po:po + HD, mq, c * P:(c + 1) * P],
                            qt[po:po + HD, mq, sh * SH:(sh + 1) * SH],
                            start=True, stop=True)
                    nc.scalar.activation(
                        pt[:, 2 * cp:2 * cp + 2, :], lg[:],
                        mybir.ActivationFunctionType.Exp)

                class AvState:
                    pass

                def av_begin(sh, h, pt, stage2=None, lane=0):
                    st = AvState()
                    st.sh, st.h, st.pt = sh, h, pt
                    st.av = psav.tile([P, QC, HD + 1], f32, tag="av")
                    st.rc = rc_pool.tile([P, QC], f32, tag="rc")
                    # stage is one lane of a shared [P, QC, 2, HD] tile so an
                    # (even, odd) head pair can share 128-wide transposes
                    if stage2 is None:
                        stage2 = stg_pool.tile([P, QC, 2, HD], bf16, tag="stg")
                    st.stage2 = stage2
                    st.stage = stage2[:, :, lane, :]
                    # transpose back into the (drained) av tile, bf16-viewed
                    st.tp = st.av[:].bitcast(bf16)
                    return st

                def av_chain(st, qc, act=False):
                    # one query-chunk of x[q, hd] (+denominator column)
                    for c in range(SC):
                        nc.tensor.matmul(
                            st.av[:, qc, :],
                            st.pt[:, c, qc * P:(qc + 1) * P],
                            vsb[:, c, st.h, :],
                            start=(c == 0), stop=(c == SC - 1))
                    if qc < QC - 1:
                        return
                    # batched epilogue: one reciprocal over the 4 denominator
                    # columns, one broadcast multiply for the normalization
                    # (fewer DVE queue entries on the drain critical path)
                    nc.vector.reciprocal(st.rc[:], st.av[:, :, HD])
                    rc_ap = st.rc[:, 0:QC]
                    rc_b = bass.AP(rc_ap.tensor, rc_ap.offset,
                                   rc_ap.ap + [[0, HD]])
                    nc.vector.tensor_mul(
                        st.stage[:], st.av[:, :, 0:HD], rc_b)

                def av_finish(st, act=False):
                    po = (st.h % 2) * HD
                    mq = st.h // 2
                    for qc in range(QC):
                        nc.tensor.transpose(
                            st.tp[0:HD, qc, 0:P], st.stage[:, qc, :],
                            ident_sb[:])
                    dst = xt[po:po + HD, mq, st.sh * SH:(st.sh + 1) * SH]
                    if act:
                        nc.scalar.copy(dst, st.tp[0:HD, :, 0:P])
                    else:
                        nc.vector.tensor_copy(dst, st.tp[0:HD, :, 0:P])

                def av_finish_pair(stE, stO):
                    # heads (2t, 2t+1) live in partition halves 0-63/64-127 of
                    # the same xt column block, so one [128,128] transpose of
                    # the shared stage handles both heads per query chunk
                    mq = stE.h // 2
                    for qc in range(QC):
                        nc.tensor.transpose(
                            stE.tp[0:P, qc, 0:P],
                            stE.stage2[:, qc, :, :], ident_sb[:])
                    nc.vector.tensor_copy(
                        xt[:, mq, stE.sh * SH:(stE.sh + 1) * SH],
                        stE.tp[0:P, :, 0:P])

                def emit_outchain(sh, m, drain=False, ob_act=False):
                    # out^T[:, half] chunk m = sum_hp wo_hp^T x_hp^T + bo.
                    # In the drain the logits pool is dead and ACT is idle, so
                    # odd chunks borrow an lg tile as accumulator and the ACT
                    # engine for the bias epilogue — a 4-deep ring instead of
                    # the ps pool ping-pong.
                    if drain and m % 2 == 1:
                        lgacc = pslg.tile([P, 2, SH], f32, tag="lg")
                        acc = lgacc[:, 0, :]
                    else:
                        psacc = ps.tile([P, SH], f32, tag="ps")
                        acc = psacc[:]
                    for hp in range(4):
                        nc.tensor.matmul(
                            acc,
                            wo_sb[:, hp, m * P:(m + 1) * P],
                            xt[:, hp, sh * SH:(sh + 1) * SH],
                            start=(hp == 0), stop=(hp == 3))
                    ob = out_pool.tile([P, SH], bf16, tag="outb")
                    if ob_act or (drain and m % 2 == 1):
                        nc.scalar.activation(
                            ob[:], acc,
                            mybir.ActivationFunctionType.Identity,
                            bias=bo_sb[:, m:m + 1])
                    else:
                        nc.vector.tensor_scalar_add(ob[:], acc, bo_sb[:, m:m + 1])
                    nc.sync.dma_start(
                        out_d[m * P:(m + 1) * P, sh * SH:(sh + 1) * SH], ob[:])

                # ---- head: K(m0) both halves, Q(sh0, m0) ----
                emit_kchain(0, kin0, 0)
                emit_kchain(1, kin1, 0)
                emit_qchain(0, qin0, 0, head=True)

                # ---- attention stream ----
                # each m-chunk of kt/qt must be projected before the first
                # logits group that reads it: kt/qt m1 by slot 2, m2 by
                # slot 4, m3 by slot 6 (sh0), and qt sh1 m-chunks by slots
                # 8/10/12/14; all of V strictly before the first AV (slot 7)
                slots = [(sh, h) for sh in range(2) for h in range(NH)]
                inject = {
                    0: [lambda: emit_kchain(0, kin0, 1),
                        lambda: emit_kchain(1, kin1, 1)],
                    1: [lambda: emit_kchain(0, kin0, 2),
                        lambda: emit_kchain(1, kin1, 2),
                        lambda: emit_qchain(0, qin0, 1)],
                    2: [lambda: emit_kchain(0, kin0, 3),
                        lambda: emit_kchain(1, kin1, 3),
                        lambda: emit_qchain(0, qin0, 2)],
                    3: [lambda: emit_vchain(0, vin0, 0),
                        lambda: emit_vchain(0, vin0, 1),
                        lambda: emit_qchain(0, qin0, 3)],
                    4: [lambda: emit_vchain(0, vin0, 2),
                        lambda: emit_vchain(0, vin0, 3)],
                    5: [lambda: emit_vchain(1, vin1, 0),
                        lambda: emit_vchain(1, vin1, 1)],
                    6: [lambda: emit_vchain(1, vin1, 2),
                        lambda: emit_vchain(1, vin1, 3)],
                }
                op0_sched = {12: [0], 13: [1], 14: [2, 3]}

                pts = {}
                av_q = []        # slots whose AV is ready to run
                av_next = 0      # next slot index whose AV has not yet run
                for i, (sh, h) in enumerate(slots):
                    pt = p_pool.tile([P, SC, SH], bf16, tag="pbuf")
                    pts[i] = pt
                    # AVs runnable this slot: V complete after slot 6's
                    # injections, and exp for slot j is complete once emitted
                    # (j < i); drain up to 2 per slot.
                    todo = []
                    if i >= 7:
                        while av_next < i and len(todo) < 2:
                            todo.append(av_next)
                            av_next += 1
                    sts = []
                    if len(todo) == 2:
                        stE = av_begin(*slots[todo[0]], pts.pop(todo[0]))
                        stO = av_begin(*slots[todo[1]], pts.pop(todo[1]),
                                       stage2=stE.stage2, lane=1)
                        sts = [stE, stO]
                    elif todo:
                        sts = [av_begin(*slots[todo[0]], pts.pop(todo[0]))]
                    emit_qk_pair(sh, h, pt, 0)
                    if sts:
                        av_chain(sts[0], 0)
                        av_chain(sts[0], 1)
                    emit_qk_pair(sh, h, pt, 1)
                    if sts:
                        av_chain(sts[0], 2)
                        av_chain(sts[0], 3)
                    emit_qk_pair(sh, h, pt, 2)
                    if len(sts) > 1:
                        av_chain(sts[1], 0)
                        av_chain(sts[1], 1)
                    elif sts:
                        av_finish(sts[0])
                    emit_qk_pair(sh, h, pt, 3)
                    if len(sts) > 1:
                        av_chain(sts[1], 2)
                        av_chain(sts[1], 3)
                        av_finish_pair(sts[0], sts[1])
                    if i == 2:
                        qin1 = emit_load(qin_d, 1)
                        wo_sb = w_pool.tile([P, 4, D], bf16, tag="w")
                        nc.sync.dma_start(
                            wo_sb[:], wo_d[:].rearrange("(k p) f -> p k f", p=P))
                    if 7 <= i <= 10:
                        emit_qchain(1, qin1, i - 7)
                    for fn in inject.get(i, ()):
                        fn()
                    # sh0 fully in x^T once slot 7's AV ran (during slot
                    # 10); the last slot's bias epilogues queue on ACT behind
                    # the final exp so DVE is free for av15's epilogue
                    for m in op0_sched.get(i, ()):
                        emit_outchain(0, m, ob_act=(i == 15 and m == 7))
                # drain: the last AV first so its DVE epilogue overlaps
                # the remaining sh0 out-projection chains on the PE, then
                # the sh1 out-projection
                while av_next < 16:
                    st = av_begin(*slots[av_next], pts.pop(av_next))
                    for qc in range(QC):
                        av_chain(st, qc)
                    av_finish(st)
                    av_next += 1
                for m in (4, 5, 6, 7):
                    emit_outchain(0, m, drain=True)
                for m in range(KC):
                    emit_outchain(1, m, drain=True)

    nc.compile()
    return nc


_program = None
_last_in_maps = None


def _get_program():
    global _program
    if _program is None:
        _program = build_program()
    return _program


def kernel(inputs_q, inputs_kv, pos_emb_q, pos_emb_k, pos_emb_v,
           key_padding_mask, wq, bq, wk, bk, wv, bv, wo, bo):
    nc = _get_program()

    wqf = np.asarray(wq, np.float32).reshape(D, H * HD)
    wkf = np.asarray(wk, np.float32).reshape(D, H * HD)
    wvf = np.asarray(wv, np.float32).reshape(D, H * HD)
    wof = np.asarray(wo, np.float32).reshape(H * HD, D)
    bqf = np.asarray(bq, np.float32).reshape(H * HD)
    bkf = np.asarray(bk, np.float32).reshape(H * HD)
    bvf = np.asarray(bv, np.float32).reshape(H * HD)
    bof = np.asarray(bo, np.float32).reshape(D)
    # bv is structurally zero in this problem; it has no cheap slot in the
    # transposed dataflow, so refuse loudly rather than silently drop it.
    assert np.all(bvf == 0.0), "nonzero bv is not supported"

    iq = np.asarray(inputs_q, np.float32)
    ikv = np.asarray(inputs_kv, np.float32)
    pqa = np.asarray(pos_emb_q, np.float32)
    pka = np.asarray(pos_emb_k, np.float32)
    pva = np.asarray(pos_emb_v, np.float32)
    mask = np.asarray(key_padding_mask, np.float32)

    scale = np.float32(1.0 / np.sqrt(HD))
    in_maps = []
    for b in range(B):
        qin_t = np.ascontiguousarray((iq[b] + pqa[b]).T.astype(bf16_np))
        kin_t = np.ascontiguousarray((ikv[b] + pka[b]).T.astype(bf16_np))
        vin_t = np.ascontiguousarray((ikv[b] + pva[b]).T.astype(bf16_np))
        mk = np.ascontiguousarray(mask[b])
        smb = np.eye(P, dtype=np.float32).astype(bf16_np)
        for hg in range(2):
            sl = slice(hg * F, (hg + 1) * F)
            bof_hg = bof if hg == 0 else np.zeros_like(bof)
            smf = np.concatenate([
                bkf[sl].reshape(4, P).T,
                (bqf[sl] * scale).reshape(4, P).T,
                bof_hg.reshape(KC, P).T,
                mk.reshape(SC, P).T,
            ], axis=1).astype(np.float32)
            in_maps.append({
                "qin": qin_t, "kin": kin_t, "vin": vin_t,
                "wq": np.ascontiguousarray(
                    (wqf[:, sl] * scale).reshape(KC, P, 4, P)
                    .transpose(1, 2, 0, 3)).astype(bf16_np),
                "wk": np.ascontiguousarray(
                    wkf[:, sl].reshape(KC, P, 4, P)
                    .transpose(1, 2, 0, 3)).astype(bf16_np),
                "wv": np.ascontiguousarray(wvf[:, sl]).astype(bf16_np),
                "wo": np.ascontiguousarray(wof[sl, :]).astype(bf16_np),
                "smf": np.ascontiguousarray(smf),
                "smb": np.ascontiguousarray(smb),
            })

    global _last_in_maps
    _last_in_maps = in_maps
    res = run_bass_kernel_spmd(nc, in_maps, list(range(2 * B)))
    outs = [res.results[i]["out_t"] for i in range(2 * B)]
    out = np.stack([(np.asarray(outs[2 * b], np.float32)
                     + np.asarray(outs[2 * b + 1], np.float32)).T
                    for b in range(B)])
    return np.ascontiguousarray(out, dtype=np.float32)
